# revision 1
# baseline (speedup 1.0000x reference)
"""Trainium2 Bass kernel for a 2-layer GCN (CascadePredictionModel).

Model (per reference):
    src/dst = edge_index + self loops; deg over dst; norm_e = rsqrt(deg[src])*rsqrt(deg[dst])
    gcn(h, W, b) = segment_sum(norm * (h@W)[src], dst) + b
    h1 = relu(gcn(x,  W1, b1))
    h2 = relu(gcn(h1, W2, b2))
    pred = noise @ W3 + b3
    out = concat([h2, pred])            # [N+M, C]

Distribution strategy (8 NeuronCores, SPMD single NEFF):
  - Destination nodes are 1D-partitioned: core k owns dst rows [1250k, 1250k+1250).
  - Feature matmul Z = h@W computed per-core for owned rows (weights replicated),
    cast to fp16, AllGather'ed into a full [10000, 512] fp16 DRAM tensor.
  - Aggregation per dst tile of 128: dma_gather pulls the (per-tile deduped,
    src-sorted) source rows into SBUF as [128, nchunk, 512]; PE accumulates
    psum += S_c^T @ G_c over chunks, where S is a host-built selection matrix
    holding the edge norms (sums parallel edges).  Bias is added with one
    identity-matmul against a broadcast-bias tile; relu on ScalarE.
  - Own-shard sources are gathered from the local pre-AllGather buffer so that
    ~1/8 of the gather volume overlaps the AllGather itself.
  - h1^T for the layer-2 matmul is built per-tile with PE transposes right
    after the layer-1 relu, so layer-2 feature matmuls pipeline behind the
    layer-1 aggregation.
  - pred rows are sharded 250/core and run inside the AllGather bubbles.

The whole per-invocation graph structure (edge sort, per-tile dedup, own/other
split, S matrices, gather indices) is built on the host; per-tile chunk counts
are maxed over cores so the single SPMD NEFF is identical on all 8 cores.
"""

import math
import time
from contextlib import ExitStack

import numpy as np

N, E, C, MPRED = 10000, 160000, 512, 2000
P = 8                 # cores
NPC = N // P          # 1250 nodes per core
TPB = 128             # dst-tile width
NT = (NPC + TPB - 1) // TPB   # 10 tiles / core (last has 98 dsts)
NPAD = NT * TPB       # 1280
PRED_PC = MPRED // P  # 250 pred rows per core
KT = C // 128         # 4 contraction tiles

_prog_cache: dict[int, tuple] = {}
LAST_RESULTS = None  # BassKernelResults of the most recent run (for test.py)


# ---------------------------------------------------------------- host tables
def _host_tables(edge_index):
    """Build per-core gather indices + selection matrices.

    Returns (NCHUNKS, idxs_list, S_list):
      NCHUNKS      : per-dst-tile chunk count (max over cores)
      idxs_list[k] : [128, NIDX//16] int16  (16-partition wrap, tiled x8)
      S_list[k]    : [128, sum(NCHUNKS), 128] fp16,
                     S[p, coff[t]+c, m] = sum of norms of edges
                     (src=u_t[c*128+p] -> dst=k*NPC+t*128+m)
    """
    ei = np.asarray(edge_index).astype(np.int64)
    src = np.concatenate([ei[0], np.arange(N, dtype=np.int64)])
    dst = np.concatenate([ei[1], np.arange(N, dtype=np.int64)])
    deg = np.bincount(dst, minlength=N).astype(np.float64)
    dis = np.where(deg > 0, 1.0 / np.sqrt(np.maximum(deg, 1.0)), 0.0)
    norm = (dis[src] * dis[dst]).astype(np.float32)

    order = np.lexsort((src, dst))
    src_s, dst_s, norm_s = src[order], dst[order], norm[order]

    # Per (core, tile): split unique srcs into "own" (this core's shard, read
    # from the local pre-AllGather buffer) and "other" (read post-AllGather).
    per_tile = []   # [(k, t, u_own_local, u_oth, pos_of_each_edge, dloc, en)]
    nown = [1] * NT
    noth = [1] * NT
    for k in range(P):
        klo, khi = k * NPC, (k + 1) * NPC
        for t in range(NT):
            lo = k * NPC + t * TPB
            hi = min(khi, lo + TPB)
            m0 = np.searchsorted(dst_s, lo)
            m1 = np.searchsorted(dst_s, hi)
            es = src_s[m0:m1]
            u = np.unique(es)
            own_mask = (u >= klo) & (u < khi)
            u_own, u_oth = u[own_mask], u[~own_mask]
            nown[t] = max(nown[t], (len(u_own) + 127) // 128)
            noth[t] = max(noth[t], (len(u_oth) + 127) // 128)
            per_tile.append((k, t, u_own, u_oth, es, dst_s[m0:m1] - lo,
                             norm_s[m0:m1]))
    NOWN, NOTH = tuple(nown), tuple(noth)
    nch = [a + b for a, b in zip(nown, noth)]
    coff = np.concatenate([[0], np.cumsum(nch)])  # chunk offset per tile
    NIDX = int(coff[-1]) * 128

    idxs_list, S_list = [], []
    for k in range(P):
        idxs = np.zeros(NIDX, dtype=np.int64)
        S = np.zeros((NIDX, TPB), dtype=np.float32)
        for (kk, t, u_own, u_oth, es, dloc, en) in per_tile[k * NT:(k + 1) * NT]:
            base = int(coff[t]) * 128          # own group first
            obase = base + NOWN[t] * 128       # then other group
            idxs[base:base + len(u_own)] = u_own - k * NPC  # local rows in zb
            idxs[obase:obase + len(u_oth)] = u_oth          # global rows in zf
            # position of each edge's src within the tile's gathered rows
            own_e = (es >= k * NPC) & (es < (k + 1) * NPC)
            pos = np.empty(len(es), dtype=np.int64)
            pos[own_e] = base + np.searchsorted(u_own, es[own_e])
            pos[~own_e] = obase + np.searchsorted(u_oth, es[~own_e])
            np.add.at(S, (pos, dloc), en)
        wrapped = np.tile(idxs.reshape(-1, 16).T, (8, 1)).astype(np.int16)
        S_host = np.ascontiguousarray(
            S.reshape(int(coff[-1]), 128, TPB).transpose(1, 0, 2)
        ).astype(np.float16)
        idxs_list.append(np.ascontiguousarray(wrapped))
        S_list.append(S_host)
    return (NOWN, NOTH), idxs_list, S_list


# ---------------------------------------------------------------- device prog
def _build_program(NCHUNKS, sim1core=False, loops=1, no_cc=False, no_gather=False,
                   nqueues=4, scratch=49152):
    """sim1core=True builds a single-core timing variant for TimelineSim:
    collectives are replaced by a DRAM->DRAM DMA of the same output size
    (close to the 8-core AllGather's wall time), everything else identical.
    loops>1 repeats the whole compute body (timing calibration: the wall-time
    slope over `loops` isolates the per-iteration device span from the
    per-execute dispatch overhead)."""
    import concourse.bacc as bacc
    import concourse.mybir as mybir
    import concourse.tile as tile

    f16, f32, i16 = mybir.dt.float16, mybir.dt.float32, mybir.dt.int16
    Relu = mybir.ActivationFunctionType.Relu
    Copy = mybir.ActivationFunctionType.Copy
    NOWN, NOTH = NCHUNKS
    COFF = [0]
    for a, b in zip(NOWN, NOTH):
        COFF.append(COFF[-1] + a + b)
    NCTOT = COFF[-1]
    NIDX = NCTOT * 128

    nc = bacc.Bacc(
        "TRN2", target_bir_lowering=False, debug=False,
        num_devices=(1 if sim1core else P),
        num_swdge_queues=nqueues,
        dynamic_dma_scratch_size=scratch,
    )

    xT_d = nc.dram_tensor("xT", [128, KT, NPAD], f16, kind="ExternalInput")
    w1_d = nc.dram_tensor("W1t", [128, KT, C], f16, kind="ExternalInput")
    w2_d = nc.dram_tensor("W2t", [128, KT, C], f16, kind="ExternalInput")
    w3_d = nc.dram_tensor("W3t", [128, KT, C], f16, kind="ExternalInput")
    s_d = nc.dram_tensor("S", [128, NCTOT, 128], f16, kind="ExternalInput")
    idx_d = nc.dram_tensor("idxs", [128, NIDX // 16], i16, kind="ExternalInput")
    bias_d = nc.dram_tensor("biasbc", [128, 3, C], f16, kind="ExternalInput")
    ident_d = nc.dram_tensor("ident", [128, 128], f16, kind="ExternalInput")
    nzT_d = nc.dram_tensor("noiseT", [128, KT, 256], f16, kind="ExternalInput")
    out_d = nc.dram_tensor("out", [NPC + PRED_PC, C], f16, kind="ExternalOutput")

    zb = [nc.dram_tensor(f"zb{l}", [NPC, C], f16, kind="Internal") for l in range(2)]
    zf = [
        nc.dram_tensor(f"zf{l}", [N, C], f16, kind="Internal",
                       addr_space=("Local" if sim1core else "Shared"))
        for l in range(2)
    ]
    zfsrc = (
        nc.dram_tensor("zfsrc", [N, C], f16, kind="Internal") if sim1core else None
    )

    with tile.TileContext(nc) as tc, ExitStack() as ctx:
        consts = ctx.enter_context(tc.tile_pool(name="consts", bufs=1))
        zpool = ctx.enter_context(tc.tile_pool(name="z", bufs=6))
        gpool = ctx.enter_context(tc.tile_pool(name="g", bufs=6))
        gown = ctx.enter_context(tc.tile_pool(name="gown", bufs=NT))
        hpool = ctx.enter_context(tc.tile_pool(name="h", bufs=3))
        opool = ctx.enter_context(tc.tile_pool(name="o", bufs=3))
        fpsum = ctx.enter_context(tc.tile_pool(name="fps", bufs=3, space="PSUM"))
        apsum = ctx.enter_context(tc.tile_pool(name="aps", bufs=3, space="PSUM"))
        tpsum = ctx.enter_context(tc.tile_pool(name="tps", bufs=2, space="PSUM"))

        xT = consts.tile([128, KT, NPAD], f16, tag="xT")
        W1 = consts.tile([128, KT, C], f16, tag="W1")
        W2 = consts.tile([128, KT, C], f16, tag="W2")
        W3 = consts.tile([128, KT, C], f16, tag="W3")
        St = consts.tile([128, NCTOT, 128], f16, tag="S")
        idxt = consts.tile([128, NIDX // 16], i16, tag="idx")
        biast = consts.tile([128, 3, C], f16, tag="bias")
        ident = consts.tile([128, 128], f16, tag="ident")
        nzT = consts.tile([128, KT, 256], f16, tag="nzT")
        h1T = consts.tile([128, KT, NPAD], f16, tag="h1T")

        # layer-1 feature operands first — S/idxs aren't needed until after
        # AllGather-1, so their big loads must not delay the first matmuls.
        nc.sync.dma_start(xT[:], xT_d[:])
        nc.sync.dma_start(W1[:], w1_d[:])
        nc.sync.dma_start(W3[:], w3_d[:])
        nc.sync.dma_start(biast[:], bias_d[:])
        nc.sync.dma_start(ident[:], ident_d[:])
        nc.sync.dma_start(nzT[:], nzT_d[:])
        nc.sync.dma_start(idxt[:], idx_d[:])
        nc.sync.dma_start(St[:], s_d[:])
        nc.sync.dma_start(W2[:], w2_d[:])

        def feature_layer(lhsT, Wt, zb_d):
            for nt in range(NT):
                ps = fpsum.tile([128, C], f32, tag="fps")
                for g in range(KT):
                    nc.tensor.matmul(
                        ps[:],
                        lhsT[:, g, nt * 128:(nt + 1) * 128],
                        Wt[:, g, :],
                        start=(g == 0),
                        stop=(g == KT - 1),
                    )
                zt = zpool.tile([128, C], f16, tag="z")
                nc.scalar.activation(zt[:], ps[:], Copy)
                w = NPC - nt * 128 if nt == NT - 1 else 128
                nc.sync.dma_start(zb_d[nt * 128: nt * 128 + w, :], zt[:w, :])

        # gather groups: <=8 chunks (1024 idxs) per dma_gather so
        # single_packet fits and gather/matmul pipelining stays fine
        def tile_groups(nch):
            ha = min((nch + 1) // 2, 8)
            return [(0, ha), (ha, nch)] if nch > ha else [(0, nch)]

        _qn = [0]

        def gather_group(src_d, t, c0, c1, pool=None, tag="g"):
            """One dma_gather of chunks [c0,c1) of tile t from src_d."""
            qn = _qn[0] % nqueues
            _qn[0] += 1
            nch = c1 - c0
            G = (pool or gpool).tile([128, nch, C], f16, tag=tag)
            if no_gather:
                nc.vector.memset(G[:, 0, 0:16], 0.0)
            else:
                nc.gpsimd.dma_gather(
                    G[:],
                    src_d[:],
                    idxt[:, COFF[t] * 8 + c0 * 8: COFF[t] * 8 + c1 * 8],
                    nch * 128,
                    nch * 128,
                    C,
                    single_packet=(nch * 128 <= 1024),
                    queue_num=qn,
                )
            return G

        def agg_own_gathers(zb_d):
            """Own-shard gathers (from the local pre-AllGather buffer) — these
            only need zb, so they run during the AllGather wait."""
            return [gather_group(zb_d, t, 0, NOWN[t], pool=gown, tag="go")
                    for t in range(NT)]

        def agg_layer(lidx, zf_d, own_G, emit_out):
            for t in range(NT):
                ps = apsum.tile([128, C], f32, tag="aps")
                for c in range(NOWN[t]):
                    nc.tensor.matmul(
                        ps[:], St[:, COFF[t] + c, :], own_G[t][:, c, :],
                        start=(c == 0), stop=False,
                    )
                for (c0, c1) in tile_groups(NOTH[t]):
                    G = gather_group(zf_d, t, NOWN[t] + c0, NOWN[t] + c1)
                    for c in range(c0, c1):
                        nc.tensor.matmul(
                            ps[:],
                            St[:, COFF[t] + NOWN[t] + c, :],
                            G[:, c - c0, :],
                            start=False,
                            stop=False,
                        )
                nc.tensor.matmul(
                    ps[:], ident[:], biast[:, lidx, :], start=False, stop=True
                )
                emit_out(t, ps)

        rg = [list(range(P))]

        def allgather(l):
            if no_cc:
                nc.sync.dma_start(zf[l][:NPC, :], zb[l][:])
            elif sim1core:
                # AllGather stand-in: just the dependency-carrying own-shard
                # copy. (Real 8-core AG wall ~14us, on collective rings that
                # don't contend with the SDMA modeled here; add ~10us/layer
                # mentally when reading sim numbers.)
                nc.sync.dma_start(zf[l][:NPC, :], zb[l][:])
            else:
                nc.gpsimd.collective_compute(
                    "AllGather",
                    bacc.mybir.AluOpType.bypass,
                    replica_groups=rg,
                    ins=[zb[l][:]],
                    outs=[zf[l][:]],
                )

        # ---- layer 1
        feature_layer(xT, W1, zb[0])

        # pred = noise @ W3 + b3 (no relu), 250 rows/core — one tile emitted in
        # each AllGather bubble so the PE has work while waiting.
        def pred_tile(mt):
            ps = fpsum.tile([128, C], f32, tag="fps")
            for g in range(KT):
                nc.tensor.matmul(
                    ps[:],
                    nzT[:, g, mt * 128:(mt + 1) * 128],
                    W3[:, g, :],
                    start=(g == 0),
                    stop=False,
                )
            nc.tensor.matmul(ps[:], ident[:], biast[:, 2, :], start=False, stop=True)
            ot = opool.tile([128, C], f16, tag="o")
            nc.scalar.activation(ot[:], ps[:], Copy)
            w = min(128, PRED_PC - mt * 128)
            nc.sync.dma_start(
                out_d[NPC + mt * 128: NPC + mt * 128 + w, :], ot[:w, :]
            )

        def l1_out(t, ps):
            # relu -> fp16, then PE-transpose the [128, 512] tile into h1T so
            # the layer-2 feature matmul for this node tile can start at once.
            ht = hpool.tile([128, C], f16, tag="h")
            nc.scalar.activation(ht[:], ps[:], Relu)
            for g in range(KT):
                pt = tpsum.tile([128, 128], f16, tag="tps")
                nc.tensor.transpose(pt[:], ht[:, g * 128:(g + 1) * 128], ident[:])
                nc.vector.tensor_copy(h1T[:, g, t * 128:(t + 1) * 128], pt[:])

        def l2_out(t, ps):
            ot = opool.tile([128, C], f16, tag="o")
            nc.scalar.activation(ot[:], ps[:], Relu)
            w = NPC - t * 128 if t == NT - 1 else 128
            nc.sync.dma_start(out_d[t * 128: t * 128 + w, :], ot[:w, :])

        for _rep in range(loops):
            feature_layer(xT, W1, zb[0])
            own1 = agg_own_gathers(zb[0])
            pred_tile(0)
            allgather(0)
            agg_layer(0, zf[0], own1, l1_out)
            # ---- layer 2
            feature_layer(h1T, W2, zb[1])
            own2 = agg_own_gathers(zb[1])
            pred_tile(1)
            allgather(1)
            agg_layer(1, zf[1], own2, l2_out)

    nc.compile()
    return nc


def _get_program(NCHUNKS):
    if NCHUNKS not in _prog_cache:
        _prog_cache[NCHUNKS] = _build_program(NCHUNKS)
    return _prog_cache[NCHUNKS]


# ---------------------------------------------------------------- entry point
def _prepare(x, edge_index, W1, b1, W2, b2, W3, b3, noise, num_missing_nodes=None,
             **_ignored):
    """Host preprocessing: returns (nc, in_maps)."""
    x = np.asarray(x, dtype=np.float32)
    W1 = np.asarray(W1, dtype=np.float32)
    W2 = np.asarray(W2, dtype=np.float32)
    W3 = np.asarray(W3, dtype=np.float32)
    b1 = np.asarray(b1, dtype=np.float32)
    b2 = np.asarray(b2, dtype=np.float32)
    b3 = np.asarray(b3, dtype=np.float32)
    noise = np.asarray(noise, dtype=np.float32)

    NCHUNKS, idxs_list, S_list = _host_tables(edge_index)
    nc = _get_program(NCHUNKS)

    def wtiles(W):
        # [512, 512] -> [128, KT, 512] fp16
        return np.ascontiguousarray(
            W.reshape(KT, 128, C).transpose(1, 0, 2)
        ).astype(np.float16)

    biasbc = np.ascontiguousarray(
        np.broadcast_to(np.stack([b1, b2, b3])[None, :, :], (128, 3, C))
    ).astype(np.float16)
    identity = np.eye(128, dtype=np.float16)
    w1t, w2t, w3t = wtiles(W1), wtiles(W2), wtiles(W3)

    in_maps = []
    for k in range(P):
        xs = np.zeros((NPAD, C), dtype=np.float16)
        xs[:NPC] = x[k * NPC:(k + 1) * NPC].astype(np.float16)
        xT = np.ascontiguousarray(
            xs.T.reshape(KT, 128, NPAD).transpose(1, 0, 2)
        )
        nz = np.zeros((256, C), dtype=np.float16)
        nz[:PRED_PC] = noise[k * PRED_PC:(k + 1) * PRED_PC].astype(np.float16)
        nzT = np.ascontiguousarray(nz.T.reshape(KT, 128, 256).transpose(1, 0, 2))
        in_maps.append({
            "xT": xT,
            "W1t": w1t,
            "W2t": w2t,
            "W3t": w3t,
            "S": S_list[k],
            "idxs": idxs_list[k],
            "biasbc": biasbc,
            "ident": identity,
            "noiseT": nzT,
        })

    return nc, in_maps


def _assemble(results):
    out = np.empty((N + MPRED, C), dtype=np.float32)
    for k in range(P):
        o = results[k]["out"].astype(np.float32)
        out[k * NPC:(k + 1) * NPC] = o[:NPC]
        out[N + k * PRED_PC: N + (k + 1) * PRED_PC] = o[NPC:NPC + PRED_PC]
    return out


def kernel(x, edge_index, W1, b1, W2, b2, W3, b3, noise, num_missing_nodes=None,
           **_ignored):
    from concourse.bass_utils import run_bass_kernel_spmd

    nc, in_maps = _prepare(x, edge_index, W1, b1, W2, b2, W3, b3, noise,
                           num_missing_nodes)
    res = run_bass_kernel_spmd(nc, in_maps, core_ids=list(range(P)))
    global LAST_RESULTS
    LAST_RESULTS = res
    return _assemble(res.results)


if __name__ == "__main__":
    t0 = time.time()
    rng = np.random.default_rng(0)
    inputs = {
        "x": rng.standard_normal((N, C), dtype=np.float32),
        "edge_index": rng.integers(0, N, (2, E)).astype(np.int32),
        "W1": rng.standard_normal((C, C), dtype=np.float32) * 0.05,
        "b1": np.zeros(C, np.float32),
        "W2": rng.standard_normal((C, C), dtype=np.float32) * 0.05,
        "b2": np.zeros(C, np.float32),
        "W3": rng.standard_normal((C, C), dtype=np.float32) * 0.05,
        "b3": np.zeros(C, np.float32),
        "noise": rng.standard_normal((MPRED, C), dtype=np.float32),
        "num_missing_nodes": MPRED,
    }
    out = kernel(**inputs)
    print("kernel done", out.shape, time.time() - t0, "s")

